# revision 7
# baseline (speedup 1.0000x reference)
"""Trainium2 Bass kernel for nn_Compressor (4-layer Perceiver compressor).

Sharding: 8 cores = 4 batch shards x 2 tensor-parallel halves.
Core c handles batch c//2 and TP half c%2 (heads t*8..t*8+8, FFN cols
t*4096..(t+1)*4096). Pairwise AllReduce (cores 2b, 2b+1) after the
attention output projection and after FFN W2.

On-device layout is fully transposed (feature dim on partitions), so no
transposes are ever needed on device:
  - latT master [d=2048 -> 16 tiles x 128p, n=512] fp32 resident in SBUF
  - xhatT (pre-normalized embeddings, host-computed) streamed per chunk
  - projections produce qT/kT [dh, seq] and v [seq, dh] directly
  - LN stats via one-pass E[x]/E[x^2] ones-matmuls
  - softmax without max-shift (|sim| < ~6), denominator via ones-matmul
Scheduling: K/V projections for layer l+1 are streamed inside layer l's
AllReduce windows (weights+xhat DMAs pre-issued), with dedicated tile
pools so no false WAR dependencies stall the PE.
Matmul operands bf16 (LN gains + attention scale folded into weights on
the host); accumulation fp32 in PSUM; residual chain fp32.
"""

import sys
import types

sys.path.insert(0, "/opt/trn_rl_repo")

import numpy as np
import ml_dtypes

BF16 = ml_dtypes.bfloat16

L, DIM, H, DH, FF = 4, 2048, 16, 128, 8192
INNER = H * DH
EPS = 1e-5
B, NLAT, S = 4, 512, 2048
TP = 2
HPC = H // TP          # 8 heads per core
CKV = HPC * DH         # 1024 kv cols per core
FFH = FF // TP         # 4096 ffn cols per core
NCORES = 8
DT = DIM // 128        # 16 d-tiles
FT = FFH // 128        # 32 f-tiles
NG = HPC // 2          # 4 head groups of 2

TRACE = False          # test.py can flip this for profiling

_cache = {}


def _install_ntff_shim():
    """antenv.axon_hooks is absent in this image; provide it so trace=True works."""
    try:
        import antenv
        if "antenv.axon_hooks" in sys.modules:
            return
        hooks = types.ModuleType("antenv.axon_hooks")
        _h = [None]
        hooks.set_axon_ntff_profile_hook = lambda h: _h.__setitem__(0, h)
        hooks.get_axon_ntff_profile_hook = lambda: _h[0]
        sys.modules["antenv.axon_hooks"] = hooks
        antenv.axon_hooks = hooks
        from trn_agent_boot.trn_boot import _ntff_profile_via_ctypes
        hk = _ntff_profile_via_ctypes("/opt/axon/libaxon_pjrt.so")
        if hk is not None:
            hooks.set_axon_ntff_profile_hook(hk)
    except Exception:
        pass


def _build(with_v_bias, with_qkb, with_fn):
    """Build the SPMD Bass program (same for every core)."""
    import concourse.bass as bass
    import concourse.tile as tile
    import concourse.mybir as mybir
    from concourse import bacc

    f32 = mybir.dt.float32
    bf16 = mybir.dt.bfloat16

    nc = bacc.Bacc("TRN2", target_bir_lowering=False, debug=False,
                   num_devices=NCORES)

    # ---- DRAM parameters (per-core shards; SPMD-identical shapes) ----
    d_xhat = nc.dram_tensor("xhat", [4, 128, 2, 8, 512], bf16, kind="ExternalInput").ap()
    d_lat0 = nc.dram_tensor("lat0", [128, DT, 512], f32, kind="ExternalInput").ap()
    d_wq = nc.dram_tensor("wq", [L, HPC, 128, DT, 128], bf16, kind="ExternalInput").ap()
    d_wk = nc.dram_tensor("wk", [L, NG, 128, DT, 256], bf16, kind="ExternalInput").ap()
    d_wv = nc.dram_tensor("wv", [L, NG, 128, DT, 256], bf16, kind="ExternalInput").ap()
    d_wo = nc.dram_tensor("wo", [L, DT, 128, HPC, 128], bf16, kind="ExternalInput").ap()
    d_w1 = nc.dram_tensor("w1", [L, FT, 128, DT, 128], bf16, kind="ExternalInput").ap()
    d_w2 = nc.dram_tensor("w2", [L, DT, 128, 2, 16, 128], bf16, kind="ExternalInput").ap()
    d_bq = nc.dram_tensor("bq", [L, 128, HPC], f32, kind="ExternalInput").ap()
    d_bk = nc.dram_tensor("bk", [L, 128, HPC], f32, kind="ExternalInput").ap()
    d_b1 = nc.dram_tensor("b1", [L, 128, FT], f32, kind="ExternalInput").ap()
    d_fng = nc.dram_tensor("fng", [128, DT], f32, kind="ExternalInput").ap()
    d_fnb = nc.dram_tensor("fnb", [128, DT], f32, kind="ExternalInput").ap()
    d_bv = None
    if with_v_bias:
        d_bv = nc.dram_tensor("bv", [L, NG, 128, 256], f32, kind="ExternalInput").ap()
    d_out = nc.dram_tensor("outT", [128, DT, 512], f32, kind="ExternalOutput").ap()

    with tile.TileContext(nc) as tc:
        with tc.tile_pool(name="pC", bufs=1) as pC, \
             tc.tile_pool(name="pLat", bufs=1) as pLat, \
             tc.tile_pool(name="pHat", bufs=1) as pHat, \
             tc.tile_pool(name="pQ", bufs=1) as pQ, \
             tc.tile_pool(name="pO", bufs=1) as pO, \
             tc.tile_pool(name="pKV", bufs=3) as pKV, \
             tc.tile_pool(name="pXh", bufs=2) as pXh, \
             tc.tile_pool(name="pEx", bufs=3) as pEx, \
             tc.tile_pool(name="pA", bufs=1) as pA, \
             tc.tile_pool(name="pW", bufs=2) as pW, \
             tc.tile_pool(name="pSm", bufs=2) as pSm, \
             tc.tile_pool(name="pStg", bufs=2) as pStg, \
             tc.tile_pool(name="psA", bufs=2, space="PSUM") as psA, \
             tc.tile_pool(name="psB", bufs=2, space="PSUM") as psB, \
             tc.tile_pool(name="psDO", bufs=1, space="PSUM") as psDO, \
             tc.tile_pool(name="psLn", bufs=2, space="PSUM") as psLn, \
             tc.tile_pool(name="pDram", bufs=4, space="DRAM") as pDram:

            Act = mybir.ActivationFunctionType
            Alu = mybir.AluOpType

            # ---- constants / whole-run residents ----
            ones_b = pC.tile([128, 128], bf16, tag="onesb")
            nc.vector.memset(ones_b, 1.0)
            bq_sb = pC.tile([128, L, HPC], f32, tag="bq")
            nc.sync.dma_start(bq_sb[:], d_bq.rearrange("l p h -> p l h"))
            bk_sb = pC.tile([128, L, HPC], f32, tag="bk")
            nc.sync.dma_start(bk_sb[:], d_bk.rearrange("l p h -> p l h"))
            b1_sb = pC.tile([128, L, FT], f32, tag="b1")
            nc.sync.dma_start(b1_sb[:], d_b1.rearrange("l p h -> p l h"))
            fng_sb = pC.tile([128, DT], f32, tag="fng")
            nc.sync.dma_start(fng_sb[:], d_fng)
            fnb_sb = pC.tile([128, DT], f32, tag="fnb")
            nc.sync.dma_start(fnb_sb[:], d_fnb)
            eps_sb = pC.tile([128, 1], f32, tag="eps")
            nc.vector.memset(eps_sb, EPS)

            latT = pLat.tile([128, DT, 512], f32, tag="lat")

            # ------------------------------------------------------------------
            # layernorm pieces (one-pass E[x], E[x^2] stats via ones-matmuls)
            # ------------------------------------------------------------------
            def ln_stats(mean_ps=None, var_ps=None, dts=range(DT)):
                if mean_ps is None:
                    mean_ps = psLn.tile([128, 512], f32, tag="cacc")
                    var_ps = psLn.tile([128, 512], f32, tag="cacc")
                for dt in dts:
                    lb = pStg.tile([128, 512], bf16, tag="lb")
                    nc.scalar.activation(lb[:], latT[:, dt, :], Act.Copy)
                    sq = pStg.tile([128, 512], bf16, tag="lb")
                    nc.vector.tensor_mul(sq[:], lb[:], lb[:])
                    nc.tensor.matmul(mean_ps[:], ones_b[:], lb[:],
                                     start=(dt == 0), stop=(dt == DT - 1))
                    nc.tensor.matmul(var_ps[:], ones_b[:], sq[:],
                                     start=(dt == 0), stop=(dt == DT - 1))
                return mean_ps, var_ps

            def ln_finalize(mean_ps, var_ps):
                mu = pSm.tile([128, 512], f32, tag="mures")
                nc.scalar.activation(mu[:], mean_ps[:], Act.Copy, scale=1.0 / DIM)
                e2 = pSm.tile([128, 512], f32, tag="tmp", bufs=3)
                nc.scalar.activation(e2[:], var_ps[:], Act.Copy, scale=1.0 / DIM)
                mu2 = pSm.tile([128, 512], f32, tag="tmp", bufs=3)
                nc.vector.tensor_mul(mu2[:], mu[:], mu[:])
                var = pSm.tile([128, 512], f32, tag="tmp", bufs=3)
                nc.vector.tensor_sub(var[:], e2[:], mu2[:])
                sd = pSm.tile([128, 512], f32, tag="tmp", bufs=3)
                nc.scalar.activation(sd[:], var[:], Act.Sqrt, bias=eps_sb[:])
                rstd = pSm.tile([128, 512], f32, tag="mures")
                nc.vector.reciprocal_approx_fast(rstd[:], sd[:])
                return mu, rstd

            def hat_center(mu):
                """hat = latT - mu (bf16); rstd applied downstream (to q)."""
                hat = pHat.tile([128, DT, 512], bf16, tag="hat")
                for dt in range(DT):
                    nc.vector.tensor_sub(hat[:, dt, :], latT[:, dt, :], mu[:])
                return hat

            def hat_norm(mu, rstd):
                """hat = (latT - mu) * rstd (bf16) for the FFN (read 32x)."""
                hat = pHat.tile([128, DT, 512], bf16, tag="hat")
                for dt in range(DT):
                    t = pStg.tile([128, 512], f32, tag="tf")
                    nc.vector.tensor_sub(t[:], latT[:, dt, :], mu[:])
                    nc.vector.tensor_mul(hat[:, dt, :], t[:], rstd[:])
                return hat

            # ------------------------------------------------------------------
            # K/V projection stream for one head group (2 heads)
            # ------------------------------------------------------------------
            def kv_load(l, g):
                """Pre-issue weight + first xhat-chunk DMAs for group g."""
                wk_t = pW.tile([128, DT, 256], bf16, tag="wkv")
                nc.sync.dma_start(wk_t[:], d_wk[l, g])
                wv_t = pW.tile([128, DT, 256], bf16, tag="wkv")
                nc.sync.dma_start(wv_t[:], d_wv[l, g])
                xh0 = pXh.tile([128, 8, 512], bf16, tag="xh")
                nc.sync.dma_start(xh0[:], d_xhat[0, :, 0])
                xh1 = pXh.tile([128, 8, 512], bf16, tag="xh")
                nc.sync.dma_start(xh1[:], d_xhat[0, :, 1])
                return (wk_t, wv_t, xh0, xh1)

            def kv_mms(l, g, w):
                wk_t, wv_t, xh0, xh1 = w
                k_sb = pKV.tile([128, 2, 4, 512], bf16, tag="k")
                v_sb = pKV.tile([128, 16, 256], bf16, tag="v")
                for sc in range(4):
                    if sc > 0:
                        xh0 = pXh.tile([128, 8, 512], bf16, tag="xh")
                        nc.sync.dma_start(xh0[:], d_xhat[sc, :, 0])
                        xh1 = pXh.tile([128, 8, 512], bf16, tag="xh")
                        nc.sync.dma_start(xh1[:], d_xhat[sc, :, 1])
                    halves = (xh0, xh1)
                    for hl in range(2):
                        kp = psA.tile([128, 512], f32, tag="aacc")
                        for dt in range(DT):
                            nc.tensor.matmul(
                                kp[:], wk_t[:, dt, hl * 128:(hl + 1) * 128],
                                halves[dt // 8][:, dt % 8, :],
                                start=(dt == 0), stop=(dt == DT - 1))
                        if with_qkb:
                            nc.scalar.activation(
                                k_sb[:, hl, sc, :], kp[:], Act.Identity,
                                bias=bk_sb[:, l, 2 * g + hl:2 * g + hl + 1])
                        else:
                            nc.vector.tensor_copy(k_sb[:, hl, sc, :], kp[:])
                    for st_ in range(4):
                        s_t = sc * 4 + st_
                        vp = psA.tile([128, 512], f32, tag="aacc")
                        for dt in range(DT):
                            nc.tensor.matmul(
                                vp[:, :256],
                                halves[dt // 8][:, dt % 8, st_ * 128:(st_ + 1) * 128],
                                wv_t[:, dt, :],
                                start=(dt == 0), stop=(dt == DT - 1))
                        if with_v_bias:
                            bvt = pStg.tile([128, 256], f32, tag="bv")
                            nc.sync.dma_start(bvt[:], d_bv[l, g])
                            nc.vector.tensor_add(v_sb[:, s_t, :],
                                                 vp[:, :256], bvt[:])
                        else:
                            nc.vector.tensor_copy(v_sb[:, s_t, :], vp[:, :256])
                return k_sb, v_sb

            # ------------------------------------------------------------------
            # attention for one head group (software-pipelined by one jt)
            # ------------------------------------------------------------------
            def attn_group(l, g, q_sb, o_sb, k_sb, v_sb):
                for hl in range(2):
                    h = 2 * g + hl
                    den = psDO.tile([128, 512], f32, tag="den")
                    op = psDO.tile([128, 512], f32, tag="op")
                    exs = []
                    for jt in range(16):
                        sc, r = jt // 4, jt % 4
                        sp = psB.tile([128, 512], f32, tag="sim")
                        nc.tensor.matmul(
                            sp[:], k_sb[:, hl, sc, r * 128:(r + 1) * 128],
                            q_sb[:, h, :], start=True, stop=True)
                        ex = pEx.tile([128, 512], bf16, tag="ex")
                        nc.scalar.activation(ex[:], sp[:], Act.Exp)
                        exs.append(ex)
                        if jt >= 1:
                            j = jt - 1
                            e = exs[j]
                            nc.tensor.matmul(den[:], ones_b[:], e[:],
                                             start=(j == 0), stop=False)
                            nc.tensor.matmul(
                                op[:], v_sb[:, j, hl * 128:(hl + 1) * 128],
                                e[:], start=(j == 0), stop=False)
                    e = exs[15]
                    nc.tensor.matmul(den[:], ones_b[:], e[:],
                                     start=False, stop=True)
                    nc.tensor.matmul(op[:], v_sb[:, 15, hl * 128:(hl + 1) * 128],
                                     e[:], start=False, stop=True)
                    rec = pSm.tile([128, 512], f32, tag="tmp", bufs=3)
                    nc.vector.reciprocal_approx_fast(rec[:], den[:])
                    nc.vector.tensor_mul(o_sb[:, h, :], op[:], rec[:])

            # ------------------------------------------------------------------
            # staged pairwise AllReduce into latT (+= reduced result)
            # ------------------------------------------------------------------
            def staged_allreduce(make_stage, chunks=1, fill=None, post_dt=None):
                csz = DT // chunks
                outs = []
                for c in range(chunks):
                    ar_in = pDram.tile([128, csz, 512], bf16, tag="ar")
                    ar_out = pDram.tile([128, csz, 512], bf16, tag="ar")
                    for i in range(csz):
                        st = make_stage(c * csz + i)
                        nc.sync.dma_start(ar_in[:, i, :], st[:])
                    nc.gpsimd.collective_compute(
                        "AllReduce", Alu.add,
                        replica_groups=[[0, 1], [2, 3], [4, 5], [6, 7]],
                        ins=[ar_in[:].opt()], outs=[ar_out[:].opt()])
                    outs.append(ar_out)
                if fill is not None:
                    fill()
                for c in range(chunks):
                    for i in range(csz):
                        dt = c * csz + i
                        st2 = pStg.tile([128, 512], bf16, tag="stg")
                        nc.sync.dma_start(st2[:], outs[c][:, i, :])
                        nc.vector.tensor_add(latT[:, dt, :], latT[:, dt, :],
                                             st2[:])
                        if post_dt is not None:
                            post_dt(dt)

            # ------------------------------------------------------------------
            # kernel body
            # ------------------------------------------------------------------
            # startup: latT per-dt chunks so LN starts as data lands
            for dt in range(DT):
                nc.sync.dma_start(latT[:, dt, :], d_lat0[:, dt, :])
            produced = {}
            preloaded = {}
            preloaded[(0, 0)] = kv_load(0, 0)

            for l in range(L):
                last = (l == L - 1)
                # ---------- attention LN + Q projection ----------
                mean_ps, var_ps = ln_stats()
                mu_a, rstd_a = ln_finalize(mean_ps, var_ps)
                hatc = hat_center(mu_a)
                q_sb = pQ.tile([128, HPC, 512], bf16, tag="q")
                for h in range(HPC):
                    wq_t = pW.tile([128, DT, 128], bf16, tag="wf")
                    nc.sync.dma_start(wq_t[:], d_wq[l, h])
                    qp = psLn.tile([128, 512], f32, tag="cacc")
                    for dt in range(DT):
                        nc.tensor.matmul(qp[:], wq_t[:, dt, :], hatc[:, dt, :],
                                         start=(dt == 0), stop=(dt == DT - 1))
                    if with_qkb:
                        qs = pStg.tile([128, 512], f32, tag="tf")
                        nc.vector.tensor_mul(qs[:], qp[:], rstd_a[:])
                        nc.scalar.activation(q_sb[:, h, :], qs[:], Act.Identity,
                                             bias=bq_sb[:, l, h:h + 1])
                    else:
                        nc.vector.tensor_mul(q_sb[:, h, :], qp[:], rstd_a[:])

                # ---------- head groups ----------
                o_sb = pO.tile([128, HPC, 512], bf16, tag="o")
                for g in range(NG):
                    kv = produced.pop((l, g), None)
                    if kv is None:
                        w = preloaded.pop((l, g), None)
                        if w is None:
                            w = kv_load(l, g)
                        kv = kv_mms(l, g, w)
                    k_sb, v_sb = kv
                    attn_group(l, g, q_sb, o_sb, k_sb, v_sb)

                # ---------- attention out projection + AllReduce ----------
                if not last:
                    preloaded[(l + 1, 0)] = kv_load(l + 1, 0)

                def wo_stage(dt2, l=l, o_sb=o_sb):
                    wo_t = pW.tile([128, HPC, 128], bf16, tag="wf")
                    nc.sync.dma_start(wo_t[:], d_wo[l, dt2])
                    yp = psA.tile([128, 512], f32, tag="aacc")
                    for ct in range(HPC):
                        nc.tensor.matmul(yp[:], wo_t[:, ct, :], o_sb[:, ct, :],
                                         start=(ct == 0), stop=(ct == HPC - 1))
                    st = pStg.tile([128, 512], bf16, tag="stg")
                    nc.vector.tensor_copy(st[:], yp[:])
                    return st

                def fill_wo(l=l):
                    if l + 1 < L:
                        w = preloaded.pop((l + 1, 0))
                        produced[(l + 1, 0)] = kv_mms(l + 1, 0, w)
                staged_allreduce(wo_stage, chunks=(2 if last else 1),
                                 fill=(None if last else fill_wo))

                # ---------- FFN ----------
                mean_ps, var_ps = ln_stats()
                mu_b, rstd_b = ln_finalize(mean_ps, var_ps)
                hat2 = hat_norm(mu_b, rstd_b)
                a_t = [pA.tile([128, 16, 512], bf16, tag=f"a{i}", name=f"a{i}")
                       for i in range(2)]
                for ft in range(FT):
                    w1_t = pW.tile([128, DT, 128], bf16, tag="wf")
                    nc.sync.dma_start(w1_t[:], d_w1[l, ft])
                    hp = psLn.tile([128, 512], f32, tag="cacc")
                    for dt in range(DT):
                        nc.tensor.matmul(hp[:], w1_t[:, dt, :], hat2[:, dt, :],
                                         start=(dt == 0), stop=(dt == DT - 1))
                    nc.scalar.activation(a_t[ft // 16][:, ft % 16, :], hp[:],
                                         Act.Silu, bias=b1_sb[:, l, ft:ft + 1])

                if not last:
                    preloaded[(l + 1, 1)] = kv_load(l + 1, 1)

                def w2_stage(dt2, l=l, a_t=a_t):
                    yp = psA.tile([128, 512], f32, tag="aacc")
                    for half in range(2):
                        w2_t = pW.tile([128, 16, 128], bf16, tag="wf")
                        nc.sync.dma_start(w2_t[:], d_w2[l, dt2, :, half])
                        for fi in range(16):
                            ft = half * 16 + fi
                            nc.tensor.matmul(yp[:], w2_t[:, fi, :],
                                             a_t[half][:, fi, :],
                                             start=(ft == 0), stop=(ft == FT - 1))
                    st = pStg.tile([128, 512], bf16, tag="stg")
                    nc.vector.tensor_copy(st[:], yp[:])
                    return st

                def fill_w2(l=l):
                    w = preloaded.pop((l + 1, 1))
                    produced[(l + 1, 1)] = kv_mms(l + 1, 1, w)
                    w = kv_load(l + 1, 2)
                    produced[(l + 1, 2)] = kv_mms(l + 1, 2, w)
                    preloaded[(l + 1, 3)] = kv_load(l + 1, 3)

                if last:
                    fstat = psLn.tile([128, 512], f32, tag="cacc")
                    fstat2 = psLn.tile([128, 512], f32, tag="cacc")

                    def final_stats(dt, mean_ps=fstat, var_ps=fstat2):
                        ln_stats(mean_ps, var_ps, dts=[dt])
                    staged_allreduce(w2_stage, chunks=4, post_dt=final_stats)
                else:
                    staged_allreduce(w2_stage, chunks=1, fill=fill_w2)

            # ---------- final layernorm (with gain/bias) + store ----------
            mu, rstd = ln_finalize(fstat, fstat2)
            for dt in range(DT):
                t1 = pStg.tile([128, 512], f32, tag="tf")
                nc.vector.tensor_sub(t1[:], latT[:, dt, :], mu[:])
                if with_fn:
                    t2 = pStg.tile([128, 512], f32, tag="tf")
                    nc.vector.tensor_mul(t2[:], t1[:], rstd[:])
                    t3 = pStg.tile([128, 512], f32, tag="tf")
                    nc.scalar.activation(t3[:], t2[:], Act.Identity,
                                         scale=fng_sb[:, dt:dt + 1],
                                         bias=fnb_sb[:, dt:dt + 1])
                else:
                    t3 = pStg.tile([128, 512], f32, tag="tf")
                    nc.vector.tensor_mul(t3[:], t1[:], rstd[:])
                nc.sync.dma_start(d_out[:, dt, :], t3[:])

    nc.compile()
    return nc


def _tile_kxm(w, kt, mt):
    """[K, M] -> [M//128 blocks][128p(K-sub), K//128, 128(M)] host layout."""
    K, M = w.shape
    return np.ascontiguousarray(
        w.reshape(K // 128, 128, M // 128, 128).transpose(2, 1, 0, 3))


def kernel(**inputs):
    inp = {k: np.asarray(v) for k, v in inputs.items()}
    latents = inp["latents"].astype(np.float32)
    seg = inp["seg_embeddings"].astype(np.float32)
    pos = inp["pos_emb"].astype(np.float32)
    nx_g, nx_b = inp["nx_g"].astype(np.float32), inp["nx_b"].astype(np.float32)
    nl_g, nl_b = inp["nl_g"].astype(np.float32), inp["nl_b"].astype(np.float32)
    Wq, Wkv, Wo = (inp["Wq"].astype(np.float32), inp["Wkv"].astype(np.float32),
                   inp["Wo"].astype(np.float32))
    fln_g, fln_b = inp["fln_g"].astype(np.float32), inp["fln_b"].astype(np.float32)
    W1, W2 = inp["W1"].astype(np.float32), inp["W2"].astype(np.float32)
    fn_g, fn_b = inp["fn_g"].astype(np.float32), inp["fn_b"].astype(np.float32)

    scale = DH ** -0.5

    # ---- host prep: normalized embeddings (input-only, layer-independent) ----
    emb = seg + pos[None, :S, :]                       # [B, S, D]
    mu = emb.mean(-1, keepdims=True)
    var = ((emb - mu) ** 2).mean(-1, keepdims=True)
    xhat = (emb - mu) / np.sqrt(var + EPS)             # [B, S, D]

    # per-core shards -------------------------------------------------------
    xhat_core = []                                     # per batch: [4,128,2,8,512] bf16
    for b in range(B):
        xT = np.ascontiguousarray(xhat[b].T)           # [D, S]
        xt = xT.reshape(DT, 128, 4, 512).transpose(2, 1, 0, 3)
        xhat_core.append(np.ascontiguousarray(
            xt.reshape(4, 128, 2, 8, 512).astype(BF16)))
    lat_core = []
    for b in range(B):
        lT = np.ascontiguousarray(latents[b].T)        # [D, N]
        lat_core.append(np.ascontiguousarray(
            lT.reshape(DT, 128, NLAT).transpose(1, 0, 2)).astype(np.float32))

    # per-TP-half weights ---------------------------------------------------
    whalf = []
    for t in range(TP):
        c0 = t * CKV
        f0 = t * FFH
        wq_l, wk_l, wv_l, wo_l, w1_l, w2_l = [], [], [], [], [], []
        bq_l, bk_l, b1_l, bv_l = [], [], [], []
        for l in range(L):
            wq_eff = (nl_g[l][:, None] * Wq[l][:, c0:c0 + CKV]) * scale
            wk_eff = nx_g[l][:, None] * Wkv[l][:, c0:c0 + CKV]
            wv_eff = nx_g[l][:, None] * Wkv[l][:, INNER + c0:INNER + c0 + CKV]
            bq = (nl_b[l] @ Wq[l][:, c0:c0 + CKV]) * scale
            bk = nx_b[l] @ Wkv[l][:, c0:c0 + CKV]
            bv = nx_b[l] @ Wkv[l][:, INNER + c0:INNER + c0 + CKV]
            w1_eff = fln_g[l][:, None] * W1[l][:, f0:f0 + FFH]
            b1 = fln_b[l] @ W1[l][:, f0:f0 + FFH]
            wq_l.append(_tile_kxm(wq_eff, DT, HPC).astype(BF16))
            # k/v grouped by head pairs: [NG][128, DT, 256]
            wk_t = wk_eff.reshape(DT, 128, NG, 256).transpose(2, 1, 0, 3)
            wv_t = wv_eff.reshape(DT, 128, NG, 256).transpose(2, 1, 0, 3)
            wk_l.append(np.ascontiguousarray(wk_t).astype(BF16))
            wv_l.append(np.ascontiguousarray(wv_t).astype(BF16))
            wo_half = Wo[l][c0:c0 + CKV, :]            # [CKV, DIM]
            wo_t = wo_half.reshape(HPC, 128, DT, 128).transpose(2, 1, 0, 3)
            wo_l.append(np.ascontiguousarray(wo_t).astype(BF16))
            w1_l.append(_tile_kxm(w1_eff, DT, FT).astype(BF16))
            w2_half = W2[l][f0:f0 + FFH, :]            # [FFH, DIM]
            w2_t = w2_half.reshape(FT, 128, DT, 128).transpose(2, 1, 0, 3)
            w2_l.append(np.ascontiguousarray(
                w2_t.reshape(DT, 128, 2, 16, 128)).astype(BF16))
            bq_l.append(np.ascontiguousarray(bq.reshape(HPC, 128).T))
            bk_l.append(np.ascontiguousarray(bk.reshape(HPC, 128).T))
            b1_l.append(np.ascontiguousarray(b1.reshape(FT, 128).T))
            bv_l.append(np.ascontiguousarray(
                np.broadcast_to(bv.reshape(NG, 1, 256), (NG, 128, 256)).copy()))
        whalf.append(dict(
            wq=np.stack(wq_l), wk=np.stack(wk_l), wv=np.stack(wv_l),
            wo=np.stack(wo_l), w1=np.stack(w1_l), w2=np.stack(w2_l),
            bq=np.stack(bq_l).astype(np.float32),
            bk=np.stack(bk_l).astype(np.float32),
            b1=np.stack(b1_l).astype(np.float32),
            bv=np.stack(bv_l).astype(np.float32)))

    fng = np.ascontiguousarray(fn_g.reshape(DT, 128).T).astype(np.float32)
    fnb = np.ascontiguousarray(fn_b.reshape(DT, 128).T).astype(np.float32)

    with_v_bias = bool(np.any(nx_b != 0.0))
    with_qkb = bool(np.any(nl_b != 0.0) or np.any(nx_b != 0.0))
    with_fn = bool(np.any(fn_g != 1.0) or np.any(fn_b != 0.0))

    _install_ntff_shim()

    key = ("nc", with_v_bias, with_qkb, with_fn)
    if key not in _cache:
        _cache[key] = _build(with_v_bias, with_qkb, with_fn)
    nc = _cache[key]

    in_maps = []
    for c in range(NCORES):
        b, t = c // 2, c % 2
        w = whalf[t]
        m = dict(xhat=xhat_core[b], lat0=lat_core[b],
                 wq=w["wq"], wk=w["wk"], wv=w["wv"], wo=w["wo"],
                 w1=w["w1"], w2=w["w2"],
                 bq=w["bq"], bk=w["bk"], b1=w["b1"],
                 fng=fng, fnb=fnb)
        if with_v_bias:
            m["bv"] = w["bv"]
        in_maps.append(m)

    from concourse.bass_utils import run_bass_kernel_spmd
    res = run_bass_kernel_spmd(nc, in_maps, list(range(NCORES)), trace=TRACE)
    if TRACE:
        kernel.last_exec_time_ns = res.exec_time_ns
        kernel.last_profile = res.profile_json

    outs = []
    for b in range(B):
        o = res.results[2 * b]["outT"]                 # [128, DT, 512]
        outT = o.transpose(1, 0, 2).reshape(DIM, NLAT)  # [D, N]
        outs.append(outT.T)                             # [N, D]
    return np.stack(outs).astype(np.float32)


# revision 14
# speedup vs baseline: 1.0347x; 1.0347x over previous
"""Trainium2 Bass kernel for nn_Compressor (4-layer Perceiver compressor).

Sharding: 8 cores = 4 batch shards x 2 tensor-parallel halves.
Core c handles batch c//2 and TP half c%2 (heads t*8..t*8+8, FFN cols
t*4096..(t+1)*4096). Pairwise AllReduce (cores 2b, 2b+1) after the
attention output projection and after FFN W2.

On-device layout is fully transposed (feature dim on partitions), so no
transposes are ever needed on device:
  - latT master [d=2048 -> 16 tiles x 128p, n=512] fp32 resident in SBUF
  - xhatT (pre-normalized embeddings, host-computed) streamed per chunk
  - projections produce qT/kT [dh, seq] and v [seq, dh] directly
  - LN stats via one-pass E[x]/E[x^2] ones-matmuls
  - softmax without max-shift (|sim| < ~6), denominator via ones-matmul
Scheduling: K/V projections for layer l+1 are streamed inside layer l's
AllReduce windows (weights+xhat DMAs pre-issued), with dedicated tile
pools so no false WAR dependencies stall the PE.
Matmul operands bf16 (LN gains + attention scale folded into weights on
the host); accumulation fp32 in PSUM; residual chain fp32.
"""

import sys
import types

sys.path.insert(0, "/opt/trn_rl_repo")

import numpy as np
import ml_dtypes

BF16 = ml_dtypes.bfloat16

L, DIM, H, DH, FF = 4, 2048, 16, 128, 8192
INNER = H * DH
EPS = 1e-5
B, NLAT, S = 4, 512, 2048
TP = 2
HPC = H // TP          # 8 heads per core
CKV = HPC * DH         # 1024 kv cols per core
FFH = FF // TP         # 4096 ffn cols per core
NCORES = 8
DT = DIM // 128        # 16 d-tiles
FT = FFH // 128        # 32 f-tiles
NG = HPC // 2          # 4 head groups of 2

TRACE = False          # test.py can flip this for profiling

_cache = {}


def _install_ntff_shim():
    """antenv.axon_hooks is absent in this image; provide it so trace=True works."""
    try:
        import antenv
        if "antenv.axon_hooks" in sys.modules:
            return
        hooks = types.ModuleType("antenv.axon_hooks")
        _h = [None]
        hooks.set_axon_ntff_profile_hook = lambda h: _h.__setitem__(0, h)
        hooks.get_axon_ntff_profile_hook = lambda: _h[0]
        sys.modules["antenv.axon_hooks"] = hooks
        antenv.axon_hooks = hooks
        from trn_agent_boot.trn_boot import _ntff_profile_via_ctypes
        hk = _ntff_profile_via_ctypes("/opt/axon/libaxon_pjrt.so")
        if hk is not None:
            hooks.set_axon_ntff_profile_hook(hk)
    except Exception:
        pass


def _build(with_v_bias, with_qkb, with_fn):
    """Build the SPMD Bass program (same for every core)."""
    import concourse.bass as bass
    import concourse.tile as tile
    import concourse.mybir as mybir
    from concourse import bacc

    f32 = mybir.dt.float32
    bf16 = mybir.dt.bfloat16

    nc = bacc.Bacc("TRN2", target_bir_lowering=False, debug=False,
                   num_devices=NCORES)

    # ---- DRAM parameters (per-core shards; SPMD-identical shapes) ----
    d_xhat = nc.dram_tensor("xhat", [4, 128, 2, 8, 512], bf16, kind="ExternalInput").ap()
    d_lat0 = nc.dram_tensor("lat0", [128, DT, 512], f32, kind="ExternalInput").ap()
    d_wq = nc.dram_tensor("wq", [L, HPC, 128, DT, 128], bf16, kind="ExternalInput").ap()
    d_wk = nc.dram_tensor("wk", [L, NG, 128, DT, 256], bf16, kind="ExternalInput").ap()
    d_wv = nc.dram_tensor("wv", [L, NG, 128, DT, 256], bf16, kind="ExternalInput").ap()
    d_wo = nc.dram_tensor("wo", [L, DT, 128, HPC, 128], bf16, kind="ExternalInput").ap()
    d_w1 = nc.dram_tensor("w1", [L, FT, 128, DT, 128], bf16, kind="ExternalInput").ap()
    d_w2 = nc.dram_tensor("w2", [L, DT, 128, 2, 16, 128], bf16, kind="ExternalInput").ap()
    d_bq = nc.dram_tensor("bq", [L, 128, HPC], f32, kind="ExternalInput").ap()
    d_bk = nc.dram_tensor("bk", [L, 128, HPC], f32, kind="ExternalInput").ap()
    d_b1 = nc.dram_tensor("b1", [L, 128, FT], f32, kind="ExternalInput").ap()
    d_fng = nc.dram_tensor("fng", [128, DT], f32, kind="ExternalInput").ap()
    d_fnb = nc.dram_tensor("fnb", [128, DT], f32, kind="ExternalInput").ap()
    d_bv = None
    if with_v_bias:
        d_bv = nc.dram_tensor("bv", [L, NG, 128, 256], f32, kind="ExternalInput").ap()
    d_out = nc.dram_tensor("outT", [128, DT, 512], f32, kind="ExternalOutput").ap()

    with tile.TileContext(nc) as tc:
        with tc.tile_pool(name="pC", bufs=1) as pC, \
             tc.tile_pool(name="pLat", bufs=1) as pLat, \
             tc.tile_pool(name="pHat", bufs=1) as pHat, \
             tc.tile_pool(name="pQ", bufs=1) as pQ, \
             tc.tile_pool(name="pKV", bufs=3) as pKV, \
             tc.tile_pool(name="pXh", bufs=3) as pXh, \
             tc.tile_pool(name="pEx", bufs=3) as pEx, \
             tc.tile_pool(name="pA", bufs=1) as pA, \
             tc.tile_pool(name="pW", bufs=2) as pW, \
             tc.tile_pool(name="pSm", bufs=2) as pSm, \
             tc.tile_pool(name="pStg", bufs=2) as pStg, \
             tc.tile_pool(name="psA", bufs=2, space="PSUM") as psA, \
             tc.tile_pool(name="psB", bufs=2, space="PSUM") as psB, \
             tc.tile_pool(name="psDO", bufs=1, space="PSUM") as psDO, \
             tc.tile_pool(name="psLn", bufs=2, space="PSUM") as psLn, \
             tc.tile_pool(name="pDram", bufs=4, space="DRAM") as pDram:

            Act = mybir.ActivationFunctionType
            Alu = mybir.AluOpType

            # ---- constants / whole-run residents ----
            ones_b = pC.tile([128, 128], bf16, tag="onesb")
            nc.vector.memset(ones_b, 1.0)
            bq_sb = pC.tile([128, L, HPC], f32, tag="bq")
            nc.sync.dma_start(bq_sb[:], d_bq.rearrange("l p h -> p l h"))
            bk_sb = pC.tile([128, L, HPC], f32, tag="bk")
            nc.sync.dma_start(bk_sb[:], d_bk.rearrange("l p h -> p l h"))
            b1_sb = pC.tile([128, L, FT], f32, tag="b1")
            nc.sync.dma_start(b1_sb[:], d_b1.rearrange("l p h -> p l h"))
            fng_sb = pC.tile([128, DT], f32, tag="fng")
            nc.sync.dma_start(fng_sb[:], d_fng)
            fnb_sb = pC.tile([128, DT], f32, tag="fnb")
            nc.sync.dma_start(fnb_sb[:], d_fnb)
            eps_sb = pC.tile([128, 1], f32, tag="eps")
            nc.vector.memset(eps_sb, EPS)

            latT = pLat.tile([128, DT, 512], f32, tag="lat")

            # ------------------------------------------------------------------
            # layernorm pieces (one-pass E[x], E[x^2] stats via ones-matmuls)
            # ------------------------------------------------------------------
            def ln_stats(mean_ps=None, var_ps=None, dts=range(DT)):
                if mean_ps is None:
                    mean_ps = psLn.tile([128, 512], f32, tag="cacc")
                    var_ps = psLn.tile([128, 512], f32, tag="cacc")
                for dt in dts:
                    lb = pStg.tile([128, 512], bf16, tag="lb")
                    nc.scalar.activation(lb[:], latT[:, dt, :], Act.Copy)
                    sq = pStg.tile([128, 512], bf16, tag="lb")
                    nc.vector.tensor_mul(sq[:], lb[:], lb[:])
                    nc.tensor.matmul(mean_ps[:], ones_b[:], lb[:],
                                     start=(dt == 0), stop=(dt == DT - 1))
                    nc.tensor.matmul(var_ps[:], ones_b[:], sq[:],
                                     start=(dt == 0), stop=(dt == DT - 1))
                return mean_ps, var_ps

            def ln_finalize(mean_ps, var_ps):
                mu = pSm.tile([128, 512], f32, tag="mures")
                nc.scalar.activation(mu[:], mean_ps[:], Act.Copy, scale=1.0 / DIM)
                e2 = pSm.tile([128, 512], f32, tag="tmp", bufs=3)
                nc.scalar.activation(e2[:], var_ps[:], Act.Copy, scale=1.0 / DIM)
                mu2 = pSm.tile([128, 512], f32, tag="tmp", bufs=3)
                nc.vector.tensor_mul(mu2[:], mu[:], mu[:])
                var = pSm.tile([128, 512], f32, tag="tmp", bufs=3)
                nc.vector.tensor_sub(var[:], e2[:], mu2[:])
                sd = pSm.tile([128, 512], f32, tag="tmp", bufs=3)
                nc.scalar.activation(sd[:], var[:], Act.Sqrt, bias=eps_sb[:])
                rstd = pSm.tile([128, 512], f32, tag="mures")
                nc.vector.reciprocal_approx_fast(rstd[:], sd[:])
                return mu, rstd

            def hat_center(mu):
                """hat = latT - mu (bf16); rstd applied downstream (to q)."""
                hat = pHat.tile([128, DT, 512], bf16, tag="hat")
                for dt in range(DT):
                    nc.vector.tensor_sub(hat[:, dt, :], latT[:, dt, :], mu[:])
                return hat

            def hat_norm(mu, rstd):
                """hat = (latT - mu) * rstd (bf16) for the FFN (read 32x)."""
                hat = pHat.tile([128, DT, 512], bf16, tag="hat")
                for dt in range(DT):
                    t = pStg.tile([128, 512], f32, tag="tf")
                    nc.vector.tensor_sub(t[:], latT[:, dt, :], mu[:])
                    nc.vector.tensor_mul(hat[:, dt, :], t[:], rstd[:])
                return hat

            # ------------------------------------------------------------------
            # K/V projection stream for one head group (2 heads)
            # ------------------------------------------------------------------
            def kv_load(l, g, with_chunks=True):
                """Pre-issue weight + first xhat-chunk DMAs for group g."""
                wk_t = pW.tile([128, DT, 256], bf16, tag="wkv")
                nc.sync.dma_start(wk_t[:], d_wk[l, g])
                wv_t = pW.tile([128, DT, 256], bf16, tag="wkv")
                nc.sync.dma_start(wv_t[:], d_wv[l, g])
                chunks = []
                if with_chunks:
                    for i in range(3):
                        ch = pXh.tile([128, 8, 512], bf16, tag="xh", name="xh")
                        nc.sync.dma_start(ch[:], d_xhat[i // 2, :, i % 2])
                        chunks.append(ch)
                return (wk_t, wv_t, chunks)

            def kv_stream(l, g, w):
                """k/v projection for head group g as a per-sc generator."""
                wk_t, wv_t, pre = w
                k_sb = pKV.tile([128, 2, 4, 512], bf16, tag="k")
                v_sb = pKV.tile([128, 16, 256], bf16, tag="v")

                def gen():
                    chunks = list(pre)
                    for sc in range(4):
                        while len(chunks) < 2 * sc + 2:
                            i = len(chunks)
                            ch = pXh.tile([128, 8, 512], bf16, tag="xh",
                                          name="xh")
                            nc.sync.dma_start(ch[:], d_xhat[i // 2, :, i % 2])
                            chunks.append(ch)
                        halves = (chunks[2 * sc], chunks[2 * sc + 1])
                        for hl in range(2):
                            kp = psA.tile([128, 512], f32, tag="aacc")
                            for dt in range(DT):
                                nc.tensor.matmul(
                                    kp[:], wk_t[:, dt, hl * 128:(hl + 1) * 128],
                                    halves[dt // 8][:, dt % 8, :],
                                    start=(dt == 0), stop=(dt == DT - 1))
                            if with_qkb:
                                nc.scalar.activation(
                                    k_sb[:, hl, sc, :], kp[:], Act.Identity,
                                    bias=bk_sb[:, l, 2 * g + hl:2 * g + hl + 1])
                            else:
                                nc.vector.tensor_copy(k_sb[:, hl, sc, :], kp[:])
                        for st_ in range(4):
                            s_t = sc * 4 + st_
                            vp = psA.tile([128, 512], f32, tag="aacc")
                            for dt in range(DT):
                                nc.tensor.matmul(
                                    vp[:, :256],
                                    halves[dt // 8][:, dt % 8, st_ * 128:(st_ + 1) * 128],
                                    wv_t[:, dt, :],
                                    start=(dt == 0), stop=(dt == DT - 1))
                            if with_v_bias:
                                bvt = pStg.tile([128, 256], f32, tag="bv")
                                nc.sync.dma_start(bvt[:], d_bv[l, g])
                                nc.vector.tensor_add(v_sb[:, s_t, :],
                                                     vp[:, :256], bvt[:])
                            else:
                                nc.vector.tensor_copy(v_sb[:, s_t, :],
                                                      vp[:, :256])
                        # prefetch next-sc chunks; safe evictions only
                        # (chunk i evicts i-3, fully read by end of this sc)
                        while len(chunks) < min(2 * sc + 5, 8):
                            i = len(chunks)
                            ch = pXh.tile([128, 8, 512], bf16, tag="xh",
                                          name="xh")
                            nc.sync.dma_start(ch[:], d_xhat[i // 2, :, i % 2])
                            chunks.append(ch)
                        yield
                return k_sb, v_sb, gen()

            def kv_mms(l, g, w):
                k_sb, v_sb, gen_ = kv_stream(l, g, w)
                for _ in gen_:
                    pass
                return k_sb, v_sb

            # ------------------------------------------------------------------
            # attention for one head group (software-pipelined by one jt)
            # ------------------------------------------------------------------
            def attn_group(l, g, q_sb, o_sb, k_sb, v_sb):
                for hl in range(2):
                    h = 2 * g + hl
                    den = psDO.tile([128, 512], f32, tag="den")
                    op = psDO.tile([128, 512], f32, tag="op")
                    exs = []
                    for jt in range(16):
                        sc, r = jt // 4, jt % 4
                        sp = psB.tile([128, 512], f32, tag="sim")
                        nc.tensor.matmul(
                            sp[:], k_sb[:, hl, sc, r * 128:(r + 1) * 128],
                            q_sb[:, h, :], start=True, stop=True)
                        ex = pEx.tile([128, 512], bf16, tag="ex")
                        nc.scalar.activation(ex[:], sp[:], Act.Exp)
                        exs.append(ex)
                        if jt >= 1:
                            j = jt - 1
                            e = exs[j]
                            nc.tensor.matmul(den[:], ones_b[:], e[:],
                                             start=(j == 0), stop=False)
                            nc.tensor.matmul(
                                op[:], v_sb[:, j, hl * 128:(hl + 1) * 128],
                                e[:], start=(j == 0), stop=False)
                    e = exs[15]
                    nc.tensor.matmul(den[:], ones_b[:], e[:],
                                     start=False, stop=True)
                    nc.tensor.matmul(op[:], v_sb[:, 15, hl * 128:(hl + 1) * 128],
                                     e[:], start=False, stop=True)
                    rec = pSm.tile([128, 512], f32, tag="tmp", bufs=3)
                    nc.vector.reciprocal_approx_fast(rec[:], den[:])
                    nc.vector.tensor_mul(o_sb[:, h, :], op[:], rec[:])

            # ------------------------------------------------------------------
            # staged pairwise AllReduce into latT (+= reduced result)
            # ------------------------------------------------------------------
            def staged_allreduce(make_stage, chunks=1, fill=None, post_dt=None):
                csz = DT // chunks
                outs = []
                for c in range(chunks):
                    ar_in = pDram.tile([128, csz, 512], bf16, tag="ar")
                    ar_out = pDram.tile([128, csz, 512], bf16, tag="ar")
                    for i in range(csz):
                        st = make_stage(c * csz + i)
                        nc.sync.dma_start(ar_in[:, i, :], st[:])
                    nc.gpsimd.collective_compute(
                        "AllReduce", Alu.add,
                        replica_groups=[[0, 1], [2, 3], [4, 5], [6, 7]],
                        ins=[ar_in[:].opt()], outs=[ar_out[:].opt()])
                    outs.append(ar_out)
                if fill is not None:
                    fill()
                for c in range(chunks):
                    for i in range(csz):
                        dt = c * csz + i
                        st2 = pStg.tile([128, 512], bf16, tag="stg")
                        nc.sync.dma_start(st2[:], outs[c][:, i, :])
                        nc.vector.tensor_add(latT[:, dt, :], latT[:, dt, :],
                                             st2[:])
                        if post_dt is not None:
                            post_dt(dt)

            # ------------------------------------------------------------------
            # kernel body
            # ------------------------------------------------------------------
            # startup: latT per-dt chunks so LN starts as data lands
            for dt in range(DT):
                nc.sync.dma_start(latT[:, dt, :], d_lat0[:, dt, :])
            produced = {}
            preloaded = {}
            preloaded[(0, 0)] = kv_load(0, 0)
            pending_stats = None       # LN stats interleaved into the prev AR

            for l in range(L):
                last = (l == L - 1)
                # kv group 2 of this layer: runs now so its MMs cover the
                # LN-A finalize + Q-weight loads right after the W2 AR.
                if l > 0:
                    w = preloaded.pop((l, 2))
                    produced[(l, 2)] = kv_mms(l, 2, w)
                # ---------- attention LN + Q projection ----------
                if pending_stats is None:
                    mean_ps, var_ps = ln_stats()
                else:
                    mean_ps, var_ps = pending_stats
                mu_a, rstd_a = ln_finalize(mean_ps, var_ps)
                hatc = hat_center(mu_a)
                q_sb = pQ.tile([128, HPC, 512], bf16, tag="q")
                for h in range(HPC):
                    wq_t = pW.tile([128, DT, 128], bf16, tag="wf")
                    nc.sync.dma_start(wq_t[:], d_wq[l, h])
                    qp = psLn.tile([128, 512], f32, tag="cacc")
                    for dt in range(DT):
                        nc.tensor.matmul(qp[:], wq_t[:, dt, :], hatc[:, dt, :],
                                         start=(dt == 0), stop=(dt == DT - 1))
                    if with_qkb:
                        qs = pStg.tile([128, 512], f32, tag="tf")
                        nc.vector.tensor_mul(qs[:], qp[:], rstd_a[:])
                        nc.scalar.activation(q_sb[:, h, :], qs[:], Act.Identity,
                                             bias=bq_sb[:, l, h:h + 1])
                    else:
                        nc.vector.tensor_mul(q_sb[:, h, :], qp[:], rstd_a[:])

                # ---------- head groups ----------
                o_sb = pHat.tile([128, HPC, 512], bf16, tag="hat", name="o_sb")
                g3_gen = None
                if l > 0:
                    w = preloaded.pop((l, 3))
                    k3, v3, g3_gen = kv_stream(l, 3, w)
                    produced[(l, 3)] = (k3, v3)
                pump = (2, 1, 1, 0)
                for g in range(NG):
                    kv = produced.pop((l, g), None)
                    if kv is None:
                        w = preloaded.pop((l, g), None)
                        if w is None:
                            w = kv_load(l, g)
                        kv = kv_mms(l, g, w)
                    k_sb, v_sb = kv
                    attn_group(l, g, q_sb, o_sb, k_sb, v_sb)
                    if g3_gen is not None:
                        for _ in range(pump[g]):
                            next(g3_gen, None)

                # ---------- attention out projection + AllReduce ----------
                if not last:
                    preloaded[(l + 1, 0)] = kv_load(l + 1, 0)

                def wo_stage(dt2, l=l, o_sb=o_sb):
                    wo_t = pW.tile([128, HPC, 128], bf16, tag="wf")
                    nc.sync.dma_start(wo_t[:], d_wo[l, dt2])
                    yp = psA.tile([128, 512], f32, tag="aacc")
                    for ct in range(HPC):
                        nc.tensor.matmul(yp[:], wo_t[:, ct, :], o_sb[:, ct, :],
                                         start=(ct == 0), stop=(ct == HPC - 1))
                    st = pStg.tile([128, 512], bf16, tag="stg")
                    nc.vector.tensor_copy(st[:], yp[:])
                    return st

                def fill_wo(l=l):
                    if l + 1 < L:
                        w = preloaded.pop((l + 1, 0))
                        produced[(l + 1, 0)] = kv_mms(l + 1, 0, w)
                statB = psLn.tile([128, 512], f32, tag="cacc", name="statB")
                statB2 = psLn.tile([128, 512], f32, tag="cacc", name="statB2")

                def stats_b(dt, m=statB, v=statB2):
                    ln_stats(m, v, dts=[dt])
                staged_allreduce(wo_stage, chunks=(2 if last else 1),
                                 fill=(None if last else fill_wo),
                                 post_dt=stats_b)

                # ---------- FFN ----------
                mu_b, rstd_b = ln_finalize(statB, statB2)
                hat2 = hat_norm(mu_b, rstd_b)
                a_t = [pA.tile([128, 16, 512], bf16, tag=f"a{i}", name=f"a{i}")
                       for i in range(2)]
                for ft in range(FT):
                    w1_t = pW.tile([128, DT, 128], bf16, tag="wf")
                    nc.sync.dma_start(w1_t[:], d_w1[l, ft])
                    hp = psLn.tile([128, 512], f32, tag="cacc")
                    for dt in range(DT):
                        nc.tensor.matmul(hp[:], w1_t[:, dt, :], hat2[:, dt, :],
                                         start=(dt == 0), stop=(dt == DT - 1))
                    nc.scalar.activation(a_t[ft // 16][:, ft % 16, :], hp[:],
                                         Act.Silu, bias=b1_sb[:, l, ft:ft + 1])

                if not last:
                    preloaded[(l + 1, 1)] = kv_load(l + 1, 1)

                def w2_stage(dt2, l=l, a_t=a_t):
                    yp = psA.tile([128, 512], f32, tag="aacc")
                    for half in range(2):
                        w2_t = pW.tile([128, 16, 128], bf16, tag="wf")
                        nc.sync.dma_start(w2_t[:], d_w2[l, dt2, :, half])
                        for fi in range(16):
                            ft = half * 16 + fi
                            nc.tensor.matmul(yp[:], w2_t[:, fi, :],
                                             a_t[half][:, fi, :],
                                             start=(ft == 0), stop=(ft == FT - 1))
                    st = pStg.tile([128, 512], bf16, tag="stg")
                    nc.vector.tensor_copy(st[:], yp[:])
                    return st

                def fill_w2(l=l):
                    w = preloaded.pop((l + 1, 1))
                    produced[(l + 1, 1)] = kv_mms(l + 1, 1, w)
                    preloaded[(l + 1, 2)] = kv_load(l + 1, 2)
                    preloaded[(l + 1, 3)] = kv_load(l + 1, 3,
                                                    with_chunks=False)

                statA = psLn.tile([128, 512], f32, tag="cacc", name="statA")
                statA2 = psLn.tile([128, 512], f32, tag="cacc", name="statA2")

                def stats_a(dt, m=statA, v=statA2):
                    ln_stats(m, v, dts=[dt])
                staged_allreduce(w2_stage, chunks=(2 if last else 1),
                                 fill=(None if last else fill_w2),
                                 post_dt=stats_a)
                pending_stats = (statA, statA2)

            # ---------- final layernorm (with gain/bias) + store ----------
            mu, rstd = ln_finalize(*pending_stats)
            for dt in range(DT):
                t1 = pStg.tile([128, 512], f32, tag="tf")
                nc.vector.tensor_sub(t1[:], latT[:, dt, :], mu[:])
                if with_fn:
                    t2 = pStg.tile([128, 512], f32, tag="tf")
                    nc.vector.tensor_mul(t2[:], t1[:], rstd[:])
                    t3 = pStg.tile([128, 512], f32, tag="tf")
                    nc.scalar.activation(t3[:], t2[:], Act.Identity,
                                         scale=fng_sb[:, dt:dt + 1],
                                         bias=fnb_sb[:, dt:dt + 1])
                else:
                    t3 = pStg.tile([128, 512], f32, tag="tf")
                    nc.vector.tensor_mul(t3[:], t1[:], rstd[:])
                nc.sync.dma_start(d_out[:, dt, :], t3[:])

    nc.compile()
    return nc


def _tile_kxm(w, kt, mt):
    """[K, M] -> [M//128 blocks][128p(K-sub), K//128, 128(M)] host layout."""
    K, M = w.shape
    return np.ascontiguousarray(
        w.reshape(K // 128, 128, M // 128, 128).transpose(2, 1, 0, 3))


def kernel(**inputs):
    inp = {k: np.asarray(v) for k, v in inputs.items()}
    latents = inp["latents"].astype(np.float32)
    seg = inp["seg_embeddings"].astype(np.float32)
    pos = inp["pos_emb"].astype(np.float32)
    nx_g, nx_b = inp["nx_g"].astype(np.float32), inp["nx_b"].astype(np.float32)
    nl_g, nl_b = inp["nl_g"].astype(np.float32), inp["nl_b"].astype(np.float32)
    Wq, Wkv, Wo = (inp["Wq"].astype(np.float32), inp["Wkv"].astype(np.float32),
                   inp["Wo"].astype(np.float32))
    fln_g, fln_b = inp["fln_g"].astype(np.float32), inp["fln_b"].astype(np.float32)
    W1, W2 = inp["W1"].astype(np.float32), inp["W2"].astype(np.float32)
    fn_g, fn_b = inp["fn_g"].astype(np.float32), inp["fn_b"].astype(np.float32)

    scale = DH ** -0.5

    # ---- host prep: normalized embeddings (input-only, layer-independent) ----
    emb = seg + pos[None, :S, :]                       # [B, S, D]
    mu = emb.mean(-1, keepdims=True)
    var = ((emb - mu) ** 2).mean(-1, keepdims=True)
    xhat = (emb - mu) / np.sqrt(var + EPS)             # [B, S, D]

    # per-core shards -------------------------------------------------------
    xhat_core = []                                     # per batch: [4,128,2,8,512] bf16
    for b in range(B):
        xT = np.ascontiguousarray(xhat[b].T)           # [D, S]
        xt = xT.reshape(DT, 128, 4, 512).transpose(2, 1, 0, 3)
        xhat_core.append(np.ascontiguousarray(
            xt.reshape(4, 128, 2, 8, 512).astype(BF16)))
    lat_core = []
    for b in range(B):
        lT = np.ascontiguousarray(latents[b].T)        # [D, N]
        lat_core.append(np.ascontiguousarray(
            lT.reshape(DT, 128, NLAT).transpose(1, 0, 2)).astype(np.float32))

    # per-TP-half weights ---------------------------------------------------
    whalf = []
    for t in range(TP):
        c0 = t * CKV
        f0 = t * FFH
        wq_l, wk_l, wv_l, wo_l, w1_l, w2_l = [], [], [], [], [], []
        bq_l, bk_l, b1_l, bv_l = [], [], [], []
        for l in range(L):
            wq_eff = (nl_g[l][:, None] * Wq[l][:, c0:c0 + CKV]) * scale
            wk_eff = nx_g[l][:, None] * Wkv[l][:, c0:c0 + CKV]
            wv_eff = nx_g[l][:, None] * Wkv[l][:, INNER + c0:INNER + c0 + CKV]
            bq = (nl_b[l] @ Wq[l][:, c0:c0 + CKV]) * scale
            bk = nx_b[l] @ Wkv[l][:, c0:c0 + CKV]
            bv = nx_b[l] @ Wkv[l][:, INNER + c0:INNER + c0 + CKV]
            w1_eff = fln_g[l][:, None] * W1[l][:, f0:f0 + FFH]
            b1 = fln_b[l] @ W1[l][:, f0:f0 + FFH]
            wq_l.append(_tile_kxm(wq_eff, DT, HPC).astype(BF16))
            # k/v grouped by head pairs: [NG][128, DT, 256]
            wk_t = wk_eff.reshape(DT, 128, NG, 256).transpose(2, 1, 0, 3)
            wv_t = wv_eff.reshape(DT, 128, NG, 256).transpose(2, 1, 0, 3)
            wk_l.append(np.ascontiguousarray(wk_t).astype(BF16))
            wv_l.append(np.ascontiguousarray(wv_t).astype(BF16))
            wo_half = Wo[l][c0:c0 + CKV, :]            # [CKV, DIM]
            wo_t = wo_half.reshape(HPC, 128, DT, 128).transpose(2, 1, 0, 3)
            wo_l.append(np.ascontiguousarray(wo_t).astype(BF16))
            w1_l.append(_tile_kxm(w1_eff, DT, FT).astype(BF16))
            w2_half = W2[l][f0:f0 + FFH, :]            # [FFH, DIM]
            w2_t = w2_half.reshape(FT, 128, DT, 128).transpose(2, 1, 0, 3)
            w2_l.append(np.ascontiguousarray(
                w2_t.reshape(DT, 128, 2, 16, 128)).astype(BF16))
            bq_l.append(np.ascontiguousarray(bq.reshape(HPC, 128).T))
            bk_l.append(np.ascontiguousarray(bk.reshape(HPC, 128).T))
            b1_l.append(np.ascontiguousarray(b1.reshape(FT, 128).T))
            bv_l.append(np.ascontiguousarray(
                np.broadcast_to(bv.reshape(NG, 1, 256), (NG, 128, 256)).copy()))
        whalf.append(dict(
            wq=np.stack(wq_l), wk=np.stack(wk_l), wv=np.stack(wv_l),
            wo=np.stack(wo_l), w1=np.stack(w1_l), w2=np.stack(w2_l),
            bq=np.stack(bq_l).astype(np.float32),
            bk=np.stack(bk_l).astype(np.float32),
            b1=np.stack(b1_l).astype(np.float32),
            bv=np.stack(bv_l).astype(np.float32)))

    fng = np.ascontiguousarray(fn_g.reshape(DT, 128).T).astype(np.float32)
    fnb = np.ascontiguousarray(fn_b.reshape(DT, 128).T).astype(np.float32)

    with_v_bias = bool(np.any(nx_b != 0.0))
    with_qkb = bool(np.any(nl_b != 0.0) or np.any(nx_b != 0.0))
    with_fn = bool(np.any(fn_g != 1.0) or np.any(fn_b != 0.0))

    _install_ntff_shim()

    key = ("nc", with_v_bias, with_qkb, with_fn)
    if key not in _cache:
        _cache[key] = _build(with_v_bias, with_qkb, with_fn)
    nc = _cache[key]

    in_maps = []
    for c in range(NCORES):
        b, t = c // 2, c % 2
        w = whalf[t]
        m = dict(xhat=xhat_core[b], lat0=lat_core[b],
                 wq=w["wq"], wk=w["wk"], wv=w["wv"], wo=w["wo"],
                 w1=w["w1"], w2=w["w2"],
                 bq=w["bq"], bk=w["bk"], b1=w["b1"],
                 fng=fng, fnb=fnb)
        if with_v_bias:
            m["bv"] = w["bv"]
        in_maps.append(m)

    from concourse.bass_utils import run_bass_kernel_spmd
    res = run_bass_kernel_spmd(nc, in_maps, list(range(NCORES)), trace=TRACE)
    if TRACE:
        kernel.last_exec_time_ns = res.exec_time_ns
        kernel.last_profile = res.profile_json

    outs = []
    for b in range(B):
        o = res.results[2 * b]["outT"]                 # [128, DT, 512]
        outT = o.transpose(1, 0, 2).reshape(DIM, NLAT)  # [D, N]
        outs.append(outT.T)                             # [N, D]
    return np.stack(outs).astype(np.float32)
